# revision 30
# baseline (speedup 1.0000x reference)
"""AxialAttention TRN2 kernel: 8-core data-parallel over the w axis.

Per core: 32 w-positions; each is an independent 256-token attention over h.

Fast path (mask all-ones, Wg == 0, so gates == bg fold into Wo):
- LN stats via bn_stats/bn_aggr on DVE; rstd via reciprocal + one Newton
  rsqrt step (no Ln/Exp activation-table thrash); LN apply as an ACT
  affine (Identity with per-partition scale/bias).
- bf16 PE transposes into a bf16 PSUM tile; single wide drain per w.
- q/k projections paired over two w-positions (512-col moving operands);
  per-head dots slice q/k tiles at partition 0/64 directly.
- Softmax denominators ride the av matmul as ones-columns in vstage;
- normalization: one wide reciprocal per head-pair + per-head multiply.
- Engine balance: DVE = LN/recip/og-mul, ACT = exp + some drains,
  Pool(gpsimd) = bias multiplies (SBUF-only), rest of drains split.
"""
import sys

sys.path.insert(0, "/opt/trn_rl_repo")

from contextlib import ExitStack

import numpy as np
import ml_dtypes

import concourse.bass as bass
import concourse.bacc as bacc
import concourse.tile as tile
from concourse import mybir
from concourse.bass_utils import run_bass_kernel_spmd

F32 = mybir.dt.float32
BF16 = mybir.dt.bfloat16
AF = mybir.ActivationFunctionType
ALU = mybir.AluOpType

B, H, W, D = 1, 256, 256, 256
HEADS, DH = 8, 64
INNER = HEADS * DH  # 512
NCORES = 8
WLOC = W // NCORES  # 32

_BUILD_CACHE = {}
USE_GPSIMD_APPLY = True
USE_GPSIMD_EBMUL = True
BF16_TRANSPOSE = True
STAGES = 5  # dev bisect: 1=LN 2=+qkproj 3=+dots/exp 4=+av/og 5=full


def _build_fast():
    key = ("fast", STAGES, BF16_TRANSPOSE, USE_GPSIMD_APPLY, USE_GPSIMD_EBMUL)
    if key in _BUILD_CACHE:
        return _BUILD_CACHE[key]

    nc = bacc.Bacc("TRN2", target_bir_lowering=False, debug=False, num_devices=NCORES)

    xw_d = nc.dram_tensor("xw", [WLOC, H, D], F32, kind="ExternalInput").ap()
    wq_d = nc.dram_tensor("wq", [2, 128, INNER], BF16, kind="ExternalInput").ap()
    wk_d = nc.dram_tensor("wk", [2, 128, INNER], BF16, kind="ExternalInput").ap()
    wv_d = nc.dram_tensor("wv", [2, 128, INNER], BF16, kind="ExternalInput").ap()
    wo_d = nc.dram_tensor("wo", [4, 128, D], BF16, kind="ExternalInput").ap()
    eb_d = nc.dram_tensor("eb", [2, 128, HEADS * H], BF16, kind="ExternalInput").ap()
    ident_d = nc.dram_tensor("ident", [128, 128], BF16 if BF16_TRANSPOSE else F32, kind="ExternalInput").ap()
    y_d = nc.dram_tensor("y", [WLOC, H, D], F32, kind="ExternalOutput").ap()

    with tile.TileContext(nc) as tc, ExitStack() as ctx:
        wp = ctx.enter_context(tc.tile_pool(name="wpool", bufs=1))
        ps = ctx.enter_context(tc.tile_pool(name="ps", bufs=1, space="PSUM"))
        xp = ctx.enter_context(tc.tile_pool(name="xp", bufs=6))
        sp = ctx.enter_context(tc.tile_pool(name="sp", bufs=3))
        qp = ctx.enter_context(tc.tile_pool(name="qp", bufs=3))
        ep = ctx.enter_context(tc.tile_pool(name="ep", bufs=3))
        op_ = ctx.enter_context(tc.tile_pool(name="op", bufs=3))

        # ---- persistent weights ----
        wq_s = [wp.tile([128, INNER], BF16, name=f"wq{k}", tag=f"wq{k}") for k in range(2)]
        wk_s = [wp.tile([128, INNER], BF16, name=f"wk{k}", tag=f"wk{k}") for k in range(2)]
        wv_s = [wp.tile([128, INNER], BF16, name=f"wv{k}", tag=f"wv{k}") for k in range(2)]
        wo_s = [wp.tile([128, D], BF16, name=f"wo{k}", tag=f"wo{k}") for k in range(4)]
        eb_s = [wp.tile([128, HEADS * H], BF16, name=f"eb{j}", tag=f"eb{j}") for j in range(2)]
        ident = wp.tile([128, 128], BF16 if BF16_TRANSPOSE else F32, name="ident", tag="ident")
        qm = [
            [
                [wp.tile([128, 512], BF16, name=f"qm{par}{m}{eo}", tag=f"qm{par}{m}{eo}") for eo in range(2)]
                for m in range(4)
            ]
            for par in range(2)
        ]
        vstage = [
            [wp.tile([128, HEADS * 128], BF16, name=f"vs{pr}{j}", tag=f"vs{pr}{j}") for j in range(2)]
            for pr in range(2)
        ]

        for k in range(2):
            nc.sync.dma_start(out=wq_s[k][:], in_=wq_d[k])
            nc.sync.dma_start(out=wk_s[k][:], in_=wk_d[k])
            nc.sync.dma_start(out=wv_s[k][:], in_=wv_d[k])
        for k in range(4):
            nc.sync.dma_start(out=wo_s[k][:], in_=wo_d[k])
        for j in range(2):
            nc.sync.dma_start(out=eb_s[j][:], in_=eb_d[j])
            for pr in range(2):
                nc.vector.memset(vstage[pr][j][:], 1.0)
        nc.sync.dma_start(out=ident[:], in_=ident_d)
        for par in range(2):
            for m in range(4):
                for eo in range(2):
                    nc.vector.memset(qm[par][m][eo][:], 0.0)

        for pi in range(WLOC // 2):
            xnt = qp.tile([128, 1024], BF16, name=f"xnt{pi}", tag="xnt")
            xnt4 = xnt.rearrange("p (k w i) -> p k w i", k=2, w=2)
            pxnt = ps.tile(
                [128, 1024], BF16 if BF16_TRANSPOSE else F32,
                name=f"pxnt{pi}", tag="pxnt", bufs=1,
            )
            pxnt4 = pxnt.rearrange("p (w k i) -> p w k i", w=2, k=2)
            for wi in range(2):
                w = 2 * pi + wi
                # ---------- load x (one DMA) ----------
                x2 = xp.tile([128, 512], F32, name=f"x{w}", tag="x2")
                nc.sync.dma_start(
                    out=x2.rearrange("p (t d) -> p t d", t=2),
                    in_=xw_d[w].rearrange("(t p) d -> p t d", t=2),
                )

                # ---------- layernorm ----------
                bns = sp.tile([128, 12], F32, name=f"bns{w}", tag="bns")
                mv = sp.tile([128, 4], F32, name=f"mv{w}", tag="mv")
                sc = sp.tile([128, 10], F32, name=f"sc{w}", tag="sc")
                for t in range(2):
                    nc.vector.bn_stats(bns[:, 6 * t : 6 * t + 6], x2[:, 256 * t : 256 * t + 256])
                    nc.vector.bn_aggr(mv[:, 2 * t : 2 * t + 2], bns[:, 6 * t : 6 * t + 6])
                mv2 = mv.rearrange("p (t c) -> p t c", c=2)
                mean2 = mv2[:, :, 0]  # [128, 2] strided
                var2 = mv2[:, :, 1]
                # rstd = rsqrt(var+eps): recip + Newton steps from y0=(1+r)/2
                nc.vector.tensor_scalar(
                    out=sc[:, 0:2], in0=var2, scalar1=0.5, scalar2=1e-5,
                    op0=ALU.mult, op1=ALU.add,
                )  # hv = 0.5*var + eps/2
                nc.vector.reciprocal(sc[:, 2:4], sc[:, 0:2])  # r = 1/hv (= 2/var)
                # seed y0 = (1 + 1/(var+eps))/2 = 0.5 + 0.25*r, good near var~1
                nc.vector.tensor_scalar(
                    out=sc[:, 4:6], in0=sc[:, 2:4], scalar1=0.25, scalar2=0.5,
                    op0=ALU.mult, op1=ALU.add,
                )  # y0 = 0.5 + 0.25*r = 0.5 + 0.5*(1/var)
                nc.vector.tensor_mul(sc[:, 6:8], sc[:, 4:6], sc[:, 4:6])  # y0^2
                nc.vector.tensor_mul(sc[:, 6:8], sc[:, 6:8], sc[:, 0:2])  # hv*y0^2
                nc.vector.tensor_scalar(
                    out=sc[:, 6:8], in0=sc[:, 6:8], scalar1=-1.0, scalar2=1.5,
                    op0=ALU.mult, op1=ALU.add,
                )  # 1.5 - 0.5*var*y0^2
                nc.vector.tensor_mul(sc[:, 4:6], sc[:, 4:6], sc[:, 6:8])  # y1
                # second Newton step for accuracy
                nc.vector.tensor_mul(sc[:, 6:8], sc[:, 4:6], sc[:, 4:6])
                nc.vector.tensor_mul(sc[:, 6:8], sc[:, 6:8], sc[:, 0:2])
                nc.vector.tensor_scalar(
                    out=sc[:, 6:8], in0=sc[:, 6:8], scalar1=-1.0, scalar2=1.5,
                    op0=ALU.mult, op1=ALU.add,
                )
                nc.vector.tensor_mul(sc[:, 4:6], sc[:, 4:6], sc[:, 6:8])  # rstd

                # LN apply: xn = (x - mean) * rstd
                xnbf = sp.tile([128, 512], BF16 if BF16_TRANSPOSE else F32, name=f"xnbf{w}", tag="xnbf")
                aeng = nc.gpsimd if USE_GPSIMD_APPLY else nc.vector
                for t in range(2):
                    aeng.tensor_scalar(
                        out=xnbf[:, 256 * t : 256 * t + 256],
                        in0=x2[:, 256 * t : 256 * t + 256],
                        scalar1=mv[:, 2 * t : 2 * t + 1],
                        scalar2=sc[:, 4 + t : 5 + t],
                        op0=ALU.subtract, op1=ALU.mult,
                    )

                if STAGES == 1:
                    ysb = sp.tile([128, 512], F32, name=f"ysbS1{w}", tag="ysb")
                    nc.vector.tensor_copy(ysb[:], xnbf[:])
                    nc.sync.dma_start(
                        out=y_d[w].rearrange("(t p) d -> p t d", t=2),
                        in_=ysb.rearrange("p (t d) -> p t d", t=2),
                    )
                    continue
                # ---------- transpose to [d, h] (bf16) ----------
                for t in range(2):
                    for k in range(2):
                        nc.tensor.transpose(
                            pxnt4[:, wi, k, 128 * t : 128 * t + 128],
                            xnbf[:, 256 * t + 128 * k : 256 * t + 128 * k + 128],
                            ident[:],
                        )

            # drains after both LNs (DVE, bf16 2x), 2 free dims per copy
            for k in range(2):
                nc.vector.tensor_copy(
                    xnt4[:, k],
                    pxnt4[:, :, k, :],
                )

            if STAGES == 1:
                continue
            # ---------- q/k projections (paired over 2 w) ----------
            par = pi % 2
            kt = []
            for which, wsb in enumerate([wq_s, wk_s]):
                for m in range(4):
                    pt = ps.tile([128, 512], F32, name=f"pp{which}{pi}_{m}", tag="pproj", bufs=2)
                    for k in range(2):
                        nc.tensor.matmul(
                            pt[:],
                            wsb[k][:, 128 * m : 128 * m + 128],
                            xnt4[:, k].rearrange("p w i -> p (w i)"),
                            start=(k == 0), stop=(k == 1),
                        )
                    if which == 0:
                        # q: two masked half-drains (zero halves persist)
                        nc.scalar.copy(qm[par][m][0][0:64, :], pt[0:64, :])
                        nc.vector.tensor_copy(qm[par][m][1][64:128, :], pt[64:128, :])
                    else:
                        st = qp.tile([128, 512], BF16, name=f"st{which}{pi}_{m}", tag=f"st{which}{m}")
                        nc.scalar.copy(st[:], pt[:])  # k drain on ACT
                        kt.append(st)

            if STAGES == 2:
                for wi in range(2):
                    w = 2 * pi + wi
                    ysb = sp.tile([128, 512], F32, name=f"ysbS2{w}", tag="ysb")
                    nc.vector.tensor_copy(ysb[:], kt[wi][:])
                    nc.sync.dma_start(
                        out=y_d[w].rearrange("(t p) d -> p t d", t=2),
                        in_=ysb.rearrange("p (t d) -> p t d", t=2),
                    )
                continue
            for wi in range(2):
                w = 2 * pi + wi

                # ---------- v projection ----------
                for ht in range(2):
                    pv = ps.tile([128, 512], F32, name=f"pv{w}_{ht}", tag="pproj", bufs=2)
                    for k in range(2):
                        nc.tensor.matmul(
                            pv[:],
                            xnt4[:, k, wi, 128 * ht : 128 * ht + 128],
                            wv_s[k][:],
                            start=(k == 0), stop=(k == 1),
                        )
                    if ht == 0:
                        nc.scalar.copy(
                            vstage[wi][ht].rearrange("p (h c) -> p h c", h=HEADS)[:, :, 0:64],
                            pv.rearrange("p (h v) -> p h v", h=HEADS),
                        )
                    else:
                        nc.vector.tensor_copy(
                            vstage[wi][ht].rearrange("p (h c) -> p h c", h=HEADS)[:, :, 0:64],
                            pv.rearrange("p (h v) -> p h v", h=HEADS),
                        )

                # ---------- dots + exp + bias mul ----------
                expd = [
                    ep.tile([128, HEADS * H], BF16, name=f"expd{w}_{j}", tag=f"expd{j}")
                    for j in range(2)
                ]
                for hpp in range(2):
                    for jt in range(2):
                        er = ep.tile([128, 1024], BF16, name=f"er{w}_{hpp}_{jt}", tag="er", bufs=3)
                        for hh in range(2):
                            hp = 2 * hpp + hh
                            pd = ps.tile([128, 512], F32, name=f"pd{w}_{hp}_{jt}", tag="pd", bufs=3)
                            for h2 in range(2):
                                h = 2 * hp + h2
                                p, eo = h // 2, h % 2
                                nc.tensor.matmul(
                                    pd[:, 256 * h2 : 256 * h2 + 256],
                                    kt[p][:, 256 * wi + 128 * jt : 256 * wi + 128 * jt + 128],
                                    qm[par][p][eo][:, 256 * wi : 256 * wi + 256],
                                    start=True, stop=True,
                                )
                            nc.scalar.activation(
                                er[:, 512 * hh : 512 * hh + 512], pd[:], AF.Exp
                            )
                        # bias multiply: 3 on Pool, 1 on DVE
                        eng = nc.vector if ((hpp == 1 and jt == 1) or not USE_GPSIMD_EBMUL) else nc.gpsimd
                        eng.tensor_mul(
                            expd[jt][:, 1024 * hpp : 1024 * hpp + 1024],
                            er[:],
                            eb_s[jt][:, 1024 * hpp : 1024 * hpp + 1024],
                        )

                if STAGES == 3:
                    ysb = sp.tile([128, 512], F32, name=f"ysbS3{w}", tag="ysb")
                    nc.vector.tensor_copy(ysb[:], expd[0][:, 0:512])
                    nc.sync.dma_start(
                        out=y_d[w].rearrange("(t p) d -> p t d", t=2),
                        in_=ysb.rearrange("p (t d) -> p t d", t=2),
                    )
                    continue
                # ---------- av + normalize (og pipelined between avs) ----------
                ogbf = [
                    op_.tile([128, H], BF16, name=f"og{w}_{hp}", tag=f"og{hp}")
                    for hp in range(4)
                ]
                pavs = [None] * 4

                def emit_av(hp, w=w, wi=wi, expd=expd, pavs=pavs):
                    pav = ps.tile([128, 512], F32, name=f"pav{w}_{hp}", tag="pavpy", bufs=2)
                    pavs[hp] = pav
                    for hh in range(2):
                        h = 2 * hp + hh
                        for jt in range(2):
                            nc.tensor.matmul(
                                pav[:, 256 * hh : 256 * hh + 256],
                                vstage[wi][jt][:, 128 * h : 128 * h + 128],
                                expd[jt][:, 256 * h : 256 * h + 256],
                                start=(jt == 0), stop=(jt == 1),
                            )

                def emit_og(hp, w=w, ogbf=ogbf, pavs=pavs):
                    pav = pavs[hp]
                    rec = op_.tile([64, 512], F32, name=f"rec{w}_{hp}", tag="rec")
                    nc.vector.reciprocal(rec[:], pav[64:128, :])
                    for hh in range(2):
                        nc.vector.tensor_tensor(
                            out=ogbf[hp][64 * hh : 64 * hh + 64, :],
                            in0=pav[0:64, 256 * hh : 256 * hh + 256],
                            in1=rec[:, 256 * hh : 256 * hh + 256],
                            op=ALU.mult,
                        )

                emit_av(0)
                emit_av(1)
                emit_og(0)
                emit_av(2)
                emit_og(1)
                emit_av(3)
                emit_og(2)
                emit_og(3)

                if STAGES == 4:
                    ysb = sp.tile([128, 512], F32, name=f"ysbS4{w}", tag="ysb")
                    nc.vector.tensor_copy(ysb[:, 0:256], ogbf[0][:])
                    nc.vector.tensor_copy(ysb[:, 256:512], ogbf[1][:])
                    nc.sync.dma_start(
                        out=y_d[w].rearrange("(t p) d -> p t d", t=2),
                        in_=ysb.rearrange("p (t d) -> p t d", t=2),
                    )
                    continue
                # ---------- out projection ----------
                py = ps.tile([128, 512], F32, name=f"py{w}", tag="pavpy", bufs=2)
                for it in range(2):
                    for kp in range(4):
                        nc.tensor.matmul(
                            py[:, 256 * it : 256 * it + 256],
                            ogbf[kp][:, 128 * it : 128 * it + 128],
                            wo_s[kp][:],
                            start=(kp == 0), stop=(kp == 3),
                        )
                ysb = sp.tile([128, 512], F32, name=f"ysb{w}", tag="ysb", bufs=3)
                nc.scalar.copy(ysb[:], py[:])
                nc.scalar.dma_start(
                    out=y_d[w].rearrange("(t p) d -> p t d", t=2),
                    in_=ysb.rearrange("p (t d) -> p t d", t=2),
                )

    nc.compile()
    _BUILD_CACHE[key] = nc
    return nc


def kernel(x, edges, mask, ln_g, ln_b, Wq, Wkv, Wo, bo, Wg, bg, We):
    x = np.asarray(x, np.float32)
    edges = np.asarray(edges, np.float32)
    mask = np.asarray(mask)
    ln_g = np.asarray(ln_g, np.float32)
    ln_b = np.asarray(ln_b, np.float32)
    Wq = np.asarray(Wq, np.float32)
    Wkv = np.asarray(Wkv, np.float32)
    Wo = np.asarray(Wo, np.float32)
    bo = np.asarray(bo, np.float32)
    Wg = np.asarray(Wg, np.float32)
    bg = np.asarray(bg, np.float32)
    We = np.asarray(We, np.float32)

    fast = bool(mask.all()) and not np.any(Wg) and not np.any(ln_b) and not np.any(bo)
    if not fast:
        return _kernel_general(x, edges, mask, ln_g, ln_b, Wq, Wkv, Wo, bo, Wg, bg, We)

    scale = DH ** -0.5
    g = ln_g[:, None]
    wq = np.ascontiguousarray((g * Wq[:, :] * scale).reshape(2, 128, INNER)).astype(ml_dtypes.bfloat16)
    wk = np.ascontiguousarray((g * Wkv[:, :INNER]).reshape(2, 128, INNER)).astype(ml_dtypes.bfloat16)
    wv = np.ascontiguousarray((g * Wkv[:, INNER:]).reshape(2, 128, INNER)).astype(ml_dtypes.bfloat16)
    wo2 = np.ascontiguousarray((bg[:, None] * Wo).reshape(4, 128, D)).astype(ml_dtypes.bfloat16)

    eb = np.einsum("ijd,dh->hij", edges[0], We)
    ebt = np.exp(eb).transpose(2, 0, 1)  # [j, h, i]
    eb_dram = np.ascontiguousarray(ebt.reshape(2, 128, HEADS * H)).astype(ml_dtypes.bfloat16)

    ident = np.eye(128, dtype=np.float32).astype(ml_dtypes.bfloat16)

    shared = dict(wq=wq, wk=wk, wv=wv, wo=wo2, eb=eb_dram, ident=ident)
    in_maps = []
    for c in range(NCORES):
        ws = slice(WLOC * c, WLOC * (c + 1))
        m = dict(shared)
        m["xw"] = np.ascontiguousarray(x[0, :, ws, :].transpose(1, 0, 2))
        in_maps.append(m)

    nc = _build_fast()
    res = run_bass_kernel_spmd(nc, in_maps, list(range(NCORES))).results

    out = np.empty((B, H, W, D), np.float32)
    for c in range(NCORES):
        out[0, :, WLOC * c : WLOC * (c + 1), :] = res[c]["y"].transpose(1, 0, 2)
    return out


# ---------------------------------------------------------------------------
# General fallback (previous-session kernel): handles mask / nonzero Wg.
# ---------------------------------------------------------------------------


def _build_general(use_mask: bool):
    key = ("gen", use_mask)
    if key in _BUILD_CACHE:
        return _BUILD_CACHE[key]

    nc = bacc.Bacc("TRN2", target_bir_lowering=False, debug=False, num_devices=NCORES)

    F32R = mybir.dt.float32r

    xw_d = nc.dram_tensor("xw", [WLOC, H, D], F32, kind="ExternalInput").ap()
    wq_d = nc.dram_tensor("wq", [D, INNER], BF16, kind="ExternalInput").ap()
    wk_d = nc.dram_tensor("wk", [D, INNER], BF16, kind="ExternalInput").ap()
    wg_d = nc.dram_tensor("wg", [D, INNER], BF16, kind="ExternalInput").ap()
    wv_d = nc.dram_tensor("wv", [D, INNER], BF16, kind="ExternalInput").ap()
    wo_d = nc.dram_tensor("wo", [INNER, D], BF16, kind="ExternalInput").ap()
    eb_d = nc.dram_tensor("eb", [2, 128, HEADS, H], BF16, kind="ExternalInput").ap()
    bgg_d = nc.dram_tensor("bgg", [128, 4], F32, kind="ExternalInput").ap()
    ident_d = nc.dram_tensor("ident", [128, 128], F32, kind="ExternalInput").ap()
    if use_mask:
        madd_d = nc.dram_tensor("madd", [WLOC, 128, 2], F32, kind="ExternalInput").ap()
    y_d = nc.dram_tensor("y", [WLOC, H, D], F32, kind="ExternalOutput").ap()

    with tile.TileContext(nc) as tc, ExitStack() as ctx:
        wp = ctx.enter_context(tc.tile_pool(name="wpool", bufs=1))
        ps = ctx.enter_context(tc.tile_pool(name="ps", bufs=1, space="PSUM"))
        xp = ctx.enter_context(tc.tile_pool(name="xp", bufs=3))
        sp = ctx.enter_context(tc.tile_pool(name="sp", bufs=3))
        qp = ctx.enter_context(tc.tile_pool(name="qp", bufs=3))
        ep = ctx.enter_context(tc.tile_pool(name="ep", bufs=3))
        op_ = ctx.enter_context(tc.tile_pool(name="op", bufs=3))

        wq_s = [wp.tile([128, INNER], BF16, name=f"wq{k}", tag=f"wq{k}") for k in range(2)]
        wk_s = [wp.tile([128, INNER], BF16, name=f"wk{k}", tag=f"wk{k}") for k in range(2)]
        wg_s = [wp.tile([128, INNER], BF16, name=f"wg{k}", tag=f"wg{k}") for k in range(2)]
        wv_s = [wp.tile([128, INNER], BF16, name=f"wv{k}", tag=f"wv{k}") for k in range(2)]
        wo_s = [wp.tile([128, D], BF16, name=f"wo{k}", tag=f"wo{k}") for k in range(4)]
        eb_s = [wp.tile([128, HEADS * H], BF16, name=f"eb{j}", tag=f"eb{j}") for j in range(2)]
        bgg_s = wp.tile([128, 4], F32, name="bgg_s", tag="bgg_s")
        ident = wp.tile([128, 128], F32, name="ident", tag="ident")
        vstage = [wp.tile([128, HEADS * 128], BF16, name=f"vstage{j}", tag=f"vstage{j}") for j in range(2)]

        for k in range(2):
            nc.sync.dma_start(out=wq_s[k][:], in_=wq_d[128 * k : 128 * k + 128, :])
            nc.sync.dma_start(out=wk_s[k][:], in_=wk_d[128 * k : 128 * k + 128, :])
            nc.sync.dma_start(out=wg_s[k][:], in_=wg_d[128 * k : 128 * k + 128, :])
            nc.sync.dma_start(out=wv_s[k][:], in_=wv_d[128 * k : 128 * k + 128, :])
        for k in range(4):
            nc.sync.dma_start(out=wo_s[k][:], in_=wo_d[128 * k : 128 * k + 128, :])
        for j in range(2):
            nc.sync.dma_start(
                out=eb_s[j][:],
                in_=eb_d[j].rearrange("p h i -> p (h i)"),
            )
        nc.sync.dma_start(out=bgg_s[:], in_=bgg_d[:])
        nc.sync.dma_start(out=ident[:], in_=ident_d[:])
        for j in range(2):
            nc.vector.memset(vstage[j][:], 1.0)

        if use_mask:
            madd_s = wp.tile([128, 2 * WLOC], F32, name="madd_s", tag="madd_s")
            nc.sync.dma_start(
                out=madd_s.rearrange("p (w j) -> p w j", w=WLOC),
                in_=madd_d.rearrange("w p j -> p w j"),
            )

        for w in range(WLOC):
            x = [xp.tile([128, D], F32, name=f"x{w}_{t}", tag=f"x{t}") for t in range(2)]
            for t in range(2):
                nc.sync.dma_start(out=x[t][:], in_=xw_d[w, 128 * t : 128 * t + 128, :])

            xn = [sp.tile([128, D], F32, name=f"xn{w}_{t}", tag=f"xn{t}") for t in range(2)]
            stats = sp.tile([128, 8], F32, name=f"st{w}", tag="st")
            scr = sp.tile([128, D], F32, name=f"scr{w}", tag="scr")
            for t in range(2):
                nc.vector.reduce_sum(stats[:, t : t + 1], x[t][:], axis=mybir.AxisListType.X)
                nc.vector.scalar_tensor_tensor(
                    out=scr[:], in0=x[t][:], scalar=1.0, in1=x[t][:],
                    op0=ALU.mult, op1=ALU.mult,
                    accum_out=stats[:, 2 + t : 3 + t],
                )
            nc.vector.tensor_scalar(
                out=stats[:, 4:6], in0=stats[:, 0:2], scalar1=1.0 / D, scalar2=None,
                op0=ALU.mult,
            )
            nc.vector.tensor_mul(scr[:, 0:2], stats[:, 4:6], stats[:, 4:6])
            nc.vector.tensor_scalar(
                out=scr[:, 2:4], in0=scr[:, 0:2], scalar1=-1.0, scalar2=1e-5,
                op0=ALU.mult, op1=ALU.add,
            )
            for t in range(2):
                nc.scalar.activation(
                    stats[:, 6 + t : 7 + t], stats[:, 2 + t : 3 + t], AF.Ln,
                    bias=scr[:, 2 + t : 3 + t], scale=1.0 / D,
                )
            nc.scalar.activation(stats[:, 6:8], stats[:, 6:8], AF.Exp, scale=-0.5)
            for t in range(2):
                nc.vector.tensor_scalar(
                    out=xn[t][:], in0=x[t][:], scalar1=stats[:, 4 + t : 5 + t],
                    scalar2=stats[:, 6 + t : 7 + t], op0=ALU.subtract, op1=ALU.mult,
                )

            pxnt = ps.tile([128, 512], F32, name=f"pxnt{w}", tag="pxnt", bufs=1)
            for t in range(2):
                for dt in range(2):
                    nc.tensor.transpose(
                        pxnt[:, 256 * dt + 128 * t : 256 * dt + 128 * t + 128],
                        xn[t][:, 128 * dt : 128 * dt + 128],
                        ident[:],
                    )
            xnt = [sp.tile([128, H], BF16, name=f"xnt{w}_{k}", tag=f"xnt{k}") for k in range(2)]
            for k in range(2):
                nc.vector.tensor_copy(xnt[k][:], pxnt[:, 256 * k : 256 * k + 256])

            qt, kt, gt = [], [], []
            for pi, (wsb, dst, pname) in enumerate(
                [(wq_s, qt, "q"), (wk_s, kt, "k"), (wg_s, gt, "g")]
            ):
                for p in range(2):
                    pt = ps.tile([128, 512], F32, name=f"pp{pname}{w}_{p}", tag="pproj", bufs=2)
                    for half in range(2):
                        m = 2 * p + half
                        for k in range(2):
                            nc.tensor.matmul(
                                pt[:, 256 * half : 256 * half + 256],
                                wsb[k][:, 128 * m : 128 * m + 128],
                                xnt[k][:],
                                start=(k == 0), stop=(k == 1),
                            )
                    if pi == 2:
                        st = qp.tile([128, 512], F32, name=f"{pname}t{w}_{p}", tag=f"{pname}t{p}")
                        nc.scalar.copy(st[:], pt[:])
                        dst.append(st)
                    else:
                        st_t = qp.tile([64, 512], BF16, name=f"{pname}tt{w}_{p}", tag=f"{pname}tt{p}")
                        st_b = qp.tile([64, 512], BF16, name=f"{pname}tb{w}_{p}", tag=f"{pname}tb{p}")
                        nc.vector.tensor_copy(st_t[:], pt[0:64, :])
                        nc.vector.tensor_copy(st_b[:], pt[64:128, :])
                        dst.append((st_t, st_b))

            for ht in range(2):
                pv = ps.tile([128, 512], F32, name=f"pv{w}_{ht}", tag="pv", bufs=1)
                for k in range(2):
                    nc.tensor.matmul(
                        pv[:],
                        xnt[k][:, 128 * ht : 128 * ht + 128],
                        wv_s[k][:],
                        start=(k == 0), stop=(k == 1),
                    )
                nc.vector.tensor_copy(
                    vstage[ht].rearrange("p (h c) -> p h c", h=HEADS)[:, :, 0:64],
                    pv.rearrange("p (h v) -> p h v", h=HEADS),
                )

            expd = [
                ep.tile([128, HEADS * H], BF16, name=f"expd{w}_{j}", tag=f"expd{j}")
                for j in range(2)
            ]
            for hp in range(4):
                for jt in range(2):
                    pd = ps.tile([128, 512], F32, name=f"pd{w}_{hp}_{jt}", tag="pdots", bufs=2)
                    for hh in range(2):
                        h = 2 * hp + hh
                        p, ch, par = h // 4, (h // 2) % 2, h % 2
                        nc.tensor.matmul(
                            pd[:, 256 * hh : 256 * hh + 256],
                            kt[p][par][:, 256 * ch + 128 * jt : 256 * ch + 128 * jt + 128],
                            qt[p][par][:, 256 * ch : 256 * ch + 256],
                            start=True, stop=True,
                        )
                    er = ep.tile([128, 512], BF16, name=f"er{w}_{hp}_{jt}", tag="eraw")
                    if use_mask:
                        nc.scalar.activation(
                            er[:], pd[:], AF.Exp,
                            bias=madd_s[:, 2 * w + jt : 2 * w + jt + 1],
                        )
                    else:
                        nc.scalar.activation(er[:], pd[:], AF.Exp)
                    nc.vector.tensor_mul(
                        expd[jt][:, 512 * hp : 512 * hp + 512],
                        er[:],
                        eb_s[jt][:, 512 * hp : 512 * hp + 512],
                    )

            ogbf = [
                op_.tile([128, H], BF16, name=f"ogbf{w}_{hp}", tag=f"ogbf{hp}")
                for hp in range(4)
            ]
            for hp in range(4):
                pav = ps.tile([128, 512], F32, name=f"pav{w}_{hp}", tag="pav", bufs=1)
                for hh in range(2):
                    h = 2 * hp + hh
                    for jt in range(2):
                        nc.tensor.matmul(
                            pav[:, 256 * hh : 256 * hh + 256],
                            vstage[jt][:, 128 * h : 128 * h + 128],
                            expd[jt][:, 256 * h : 256 * h + 256],
                            start=(jt == 0), stop=(jt == 1),
                        )
                og1 = op_.tile([128, H], F32, name=f"og1{w}_{hp}", tag=f"og1{hp}")
                rec = op_.tile([128, H], F32, name=f"rec{w}_{hp}", tag=f"rec{hp}")
                for hh in range(2):
                    h = 2 * hp + hh
                    mt, ro = h // 2, (h % 2) * 64
                    nc.vector.scalar_tensor_tensor(
                        out=og1[64 * hh : 64 * hh + 64, :],
                        in0=gt[mt // 2][ro : ro + 64, 256 * (mt % 2) : 256 * (mt % 2) + 256],
                        scalar=bgg_s[ro : ro + 64, mt : mt + 1],
                        in1=pav[0:64, 256 * hh : 256 * hh + 256],
                        op0=ALU.add, op1=ALU.mult,
                    )
                    nc.vector.reciprocal(
                        rec[64 * hh : 64 * hh + 64, :],
                        pav[64:128, 256 * hh : 256 * hh + 256],
                    )
                nc.vector.tensor_mul(ogbf[hp][:], og1[:], rec[:])

            py = ps.tile([128, 512], F32, name=f"py{w}", tag="py", bufs=1)
            for it in range(2):
                for kp in range(4):
                    nc.tensor.matmul(
                        py[:, 256 * it : 256 * it + 256],
                        ogbf[kp][:, 128 * it : 128 * it + 128],
                        wo_s[kp][:],
                        start=(kp == 0), stop=(kp == 3),
                    )
            ysb = sp.tile([128, 512], F32, name=f"ysb{w}", tag="ysb")
            nc.scalar.copy(ysb[:], py[:])
            for it in range(2):
                nc.sync.dma_start(
                    out=y_d[w, 128 * it : 128 * it + 128, :],
                    in_=ysb[:, 256 * it : 256 * it + 256],
                )

    nc.compile()
    _BUILD_CACHE[key] = nc
    return nc


def _kernel_general(x, edges, mask, ln_g, ln_b, Wq, Wkv, Wo, bo, Wg, bg, We):
    assert not np.any(ln_b) and not np.any(bo), "ln_b/bo folding not emitted"
    scale = DH ** -0.5
    g = ln_g[:, None]
    wq = np.ascontiguousarray(g * Wq[:, :] * scale).astype(ml_dtypes.bfloat16)
    wk = np.ascontiguousarray(g * Wkv[:, :INNER]).astype(ml_dtypes.bfloat16)
    wv = np.ascontiguousarray(g * Wkv[:, INNER:]).astype(ml_dtypes.bfloat16)
    wg = np.ascontiguousarray(g * Wg).astype(ml_dtypes.bfloat16)
    wo = Wo.astype(ml_dtypes.bfloat16)
    bgg = np.ascontiguousarray(bg.reshape(4, 128).T)

    eb = np.einsum("ijd,dh->hij", edges[0], We)
    ebt = np.exp(eb).transpose(2, 0, 1)
    eb_dram = np.ascontiguousarray(ebt.reshape(2, 128, HEADS, H)).astype(ml_dtypes.bfloat16)

    ident = np.eye(128, dtype=np.float32)
    use_mask = not bool(mask.all())

    shared = dict(wq=wq, wk=wk, wg=wg, wv=wv, wo=wo, eb=eb_dram, bgg=bgg, ident=ident)
    in_maps = []
    for c in range(NCORES):
        ws = slice(WLOC * c, WLOC * (c + 1))
        m = dict(shared)
        m["xw"] = np.ascontiguousarray(x[0, :, ws, :].transpose(1, 0, 2))
        if use_mask:
            mw = (~mask[0, :, ws].T.astype(bool)).astype(np.float32) * -1e30
            m["madd"] = np.ascontiguousarray(mw.reshape(WLOC, 2, 128).transpose(0, 2, 1))
        in_maps.append(m)

    nc = _build_general(use_mask)
    res = run_bass_kernel_spmd(nc, in_maps, list(range(NCORES))).results

    out = np.empty((B, H, W, D), np.float32)
    for c in range(NCORES):
        out[0, :, WLOC * c : WLOC * (c + 1), :] = res[c]["y"].transpose(1, 0, 2)
    return out


if __name__ == "__main__":
    import reference

    inputs = {k: np.asarray(v) for k, v in reference.setup_inputs().items()}
    got = kernel(**inputs)
    exp = np.asarray(reference.reference(**inputs))
    err = np.abs(got - exp).max() / (np.abs(exp).max() + 1e-30)
    rel = np.linalg.norm(got - exp) / np.linalg.norm(exp)
    print("absmax-rel:", err, "l2-rel:", rel)
